# revision 7
# baseline (speedup 1.0000x reference)
"""Trainium2 Bass kernel for nn_FilmLayerNetwork.

Sharding: one NeuronCore per SMAB processor (NPROC = 8 = n_cores).
Each core computes its processor's full 512-map output slice:
  - stage-0 projections (K@Wq1 etc.) as k-chunked matmuls over the 512 maps
  - MHA1 (3 heads, d=32) / fc1 / MHA2 (16 heads, d=6, via segment-indicator
    matmuls) / fc2 / Wo projection
  - alpha = sigmoid(x@Wa+ba) and de = (De*gate).sum(-1) computed per-core
    (Wa replicated), final FiLM mix done on-device in a (128,4) layout.
Host packs each core's inputs into a few contiguous buffers (so DMAs are
large and contiguous), and transposes the (128,4) per-core result back.
"""

import numpy as np

NM, ZG, HID, SEQ = 512, 512, 96, 8
H1, H2, NPROC, NB = 3, 16, 8, 2
SCL = float(1.0 / np.sqrt(96.0))

# b96 column layout
C_F1, C_WQ2, C_WK2, C_WV2, C_F2, C_WO, C_E, C_BQR = (
    0, 96, 192, 288, 384, 480, 992, 1008)
B96_COLS = 1009
# b8 column layout: De_iT | gateT | I8
C_DE, C_GATE, C_I8 = 0, 512, 513
B8_COLS = 521

_CACHE = {}


def _build_nc():
    import concourse.bacc as bacc
    import concourse.tile as tile
    import concourse.mybir as mybir

    f32 = mybir.dt.float32
    AX = mybir.AxisListType
    ALU = mybir.AluOpType
    ACT = mybir.ActivationFunctionType

    nc = bacc.Bacc("TRN2", target_bir_lowering=False, debug=False, num_devices=NPROC)

    d_px = nc.dram_tensor("px", [128, 36], f32, kind="ExternalInput").ap()
    d_w1 = nc.dram_tensor("w1", [128, 1152], f32, kind="ExternalInput").ap()
    d_b96 = nc.dram_tensor("b96", [96, B96_COLS], f32, kind="ExternalInput").ap()
    d_wqr = nc.dram_tensor("wqr", [128, 384], f32, kind="ExternalInput").ap()
    d_b8 = nc.dram_tensor("b8", [8, B8_COLS], f32, kind="ExternalInput").ap()
    d_b16 = nc.dram_tensor("b16", [16, 96], f32, kind="ExternalInput").ap()
    d_t128 = nc.dram_tensor("t128", [128, 9], f32, kind="ExternalInput").ap()
    d_wa = nc.dram_tensor("wa", [128, 2048], f32, kind="ExternalInput").ap()
    d_out = nc.dram_tensor("out", [128, 4], f32, kind="ExternalOutput").ap()

    with tile.TileContext(nc) as tc, \
         tc.tile_pool(name="sb", bufs=1) as sb, \
         tc.tile_pool(name="ps", bufs=8, space="PSUM") as ps:

        def sbt(shape, tag):
            return sb.tile(shape, f32, tag=tag, name=tag)

        def pst(shape, tag):
            return ps.tile(shape, f32, tag="ps_shared", name=tag)

        # ---- input DMAs (ordered roughly by when they're needed) ----
        sb_px = sbt([128, 36], "sb_px")
        nc.sync.dma_start(out=sb_px[:], in_=d_px[:])
        sb_w1 = sbt([128, 1152], "sb_w1")
        nc.sync.dma_start(out=sb_w1[:], in_=d_w1[:])
        sb_96 = sbt([96, B96_COLS], "sb_96")
        nc.sync.dma_start(out=sb_96[:], in_=d_b96[:])
        sb_wqr = sbt([128, 384], "sb_wqr")
        nc.sync.dma_start(out=sb_wqr[:], in_=d_wqr[:])
        sb_8 = sbt([8, B8_COLS], "sb_8")
        nc.sync.dma_start(out=sb_8[:], in_=d_b8[:])
        sb_16 = sbt([16, 96], "sb_16")
        nc.sync.dma_start(out=sb_16[:], in_=d_b16[:])
        sb_t = sbt([128, 9], "sb_t")
        nc.sync.dma_start(out=sb_t[:], in_=d_t128[:])
        sb_wa = sbt([128, 2048], "sb_wa")
        nc.sync.dma_start(out=sb_wa[:], in_=d_wa[:])

        P_blk = lambda k: sb_px[:, 8 * k:8 * k + 8]
        xT_blk = lambda k: sb_px[:, 32 + k:33 + k]

        # ---- stage 0: K@wq1 (both orientations), K@wk1 (T), K@wv1, qT ----
        ps_qkT = pst([96, 8], "ps_qkT")
        ps_kkT = pst([96, 8], "ps_kkT")
        ps_vv = pst([8, 96], "ps_vv")
        ps_qT = pst([96, 1], "ps_qT")
        for k in range(4):
            s, e = k == 0, k == 3
            wq1 = sb_w1[:, 96 * k:96 * k + 96]
            wk1 = sb_w1[:, 384 + 96 * k:384 + 96 * k + 96]
            wv1 = sb_w1[:, 768 + 96 * k:768 + 96 * k + 96]
            nc.tensor.matmul(ps_qkT[:], wq1, P_blk(k), start=s, stop=e)
            nc.tensor.matmul(ps_kkT[:], wk1, P_blk(k), start=s, stop=e)
            nc.tensor.matmul(ps_vv[:], P_blk(k), wv1, start=s, stop=e)
            nc.tensor.matmul(ps_qT[:], sb_wqr[:, 96 * k:96 * k + 96], xT_blk(k),
                             start=s, stop=e)

        sb_qkT = sbt([96, 8], "sb_qkT")
        nc.scalar.copy(sb_qkT[:], ps_qkT[:])
        sb_kkT = sbt([96, 8], "sb_kkT")
        nc.scalar.copy(sb_kkT[:], ps_kkT[:])
        sb_vv = sbt([8, 96], "sb_vv")
        nc.scalar.copy(sb_vv[:], ps_vv[:])
        sb_qT = sbt([96, 1], "sb_qT")
        nc.scalar.activation(sb_qT[:], ps_qT[:], ACT.Relu,
                             bias=sb_96[:, C_BQR:C_BQR + 1])

        # ---- MHA1: 3 heads, d=32, softmax over keys (free dim) ----
        # All matmuls stay at PE tile position (0,0): per-head contractions
        # use a zero-masked lhsT over the full 96-feature partition range
        # instead of partition-offset operands (avoids cross-tile PSUM-bank
        # hazards), and the A->A^T transpose runs on the DVE 32x32 stream
        # transposer instead of PE transpose (illegal with column tiling).
        sb_qkTm = sbt([96, 24], "sb_qkTm")
        nc.vector.memset(sb_qkTm[:], 0.0)
        for h in range(3):
            nc.scalar.copy(sb_qkTm[32 * h:32 * h + 32, 8 * h:8 * h + 8],
                           sb_qkT[32 * h:32 * h + 32, :])
        ps_s = pst([8, 24], "ps_s")
        for h in range(3):
            nc.tensor.matmul(ps_s[:, 8 * h:8 * h + 8],
                             sb_qkTm[:, 8 * h:8 * h + 8], sb_kkT[:])
        sb_pexp = sbt([8, 24], "sb_pexp")
        nc.scalar.activation(sb_pexp[:], ps_s[:], ACT.Exp, scale=SCL)
        sb_sums = sbt([8, 3], "sb_sums")
        nc.vector.tensor_reduce(sb_sums[:],
                                sb_pexp.rearrange("p (h k) -> p h k", h=3),
                                AX.X, ALU.add)
        sb_rec = sbt([8, 3], "sb_rec")
        nc.vector.reciprocal(sb_rec[:], sb_sums[:])
        sb_a32 = sbt([32, 96], "sb_a32")
        nc.vector.memset(sb_a32[:], 0.0)
        for h in range(3):
            nc.scalar.mul(sb_a32[0:8, 32 * h:32 * h + 8],
                          sb_pexp[:, 8 * h:8 * h + 8], sb_rec[:, h:h + 1])
        sb_aT32 = sbt([32, 96], "sb_aT32")
        nc.vector.transpose(sb_aT32[:], sb_a32[:])

        def A_T(h):
            return sb_aT32[0:8, 32 * h:32 * h + 8]

        sb_vm = sbt([8, 288], "sb_vm")
        nc.vector.memset(sb_vm[:], 0.0)
        for h in range(3):
            nc.scalar.copy(sb_vm[:, 128 * h:128 * h + 32],
                           sb_vv[:, 32 * h:32 * h + 32])
        ps_oT = pst([96, 8], "ps_oT")
        for h in range(3):
            nc.tensor.matmul(ps_oT[:], sb_vm[:, 96 * h:96 * h + 96], A_T(h),
                             start=(h == 0), stop=(h == 2))
        sb_hT = sbt([96, 8], "sb_hT")
        nc.vector.tensor_add(sb_hT[:], ps_oT[:], sb_qkT[:])

        # ---- fc1 residual (transposed orientation only) ----
        ps_rT = pst([96, 8], "ps_rT")
        nc.tensor.matmul(ps_rT[:], sb_96[:, C_F1:C_F1 + 96], sb_hT[:])
        sb_rT = sbt([96, 8], "sb_rT")
        nc.scalar.activation(sb_rT[:], ps_rT[:], ACT.Relu)
        sb_h2T = sbt([96, 8], "sb_h2T")
        nc.vector.tensor_add(sb_h2T[:], sb_hT[:], sb_rT[:])

        # ---- stage 2: task query attends to the set (16 heads, d=6) ----
        ps_k2T = pst([96, 8], "ps_k2T")
        nc.tensor.matmul(ps_k2T[:], sb_96[:, C_WK2:C_WK2 + 96], sb_h2T[:])
        ps_v2T = pst([96, 8], "ps_v2T")
        nc.tensor.matmul(ps_v2T[:], sb_96[:, C_WV2:C_WV2 + 96], sb_h2T[:])
        ps_qqT = pst([96, 1], "ps_qqT")
        nc.tensor.matmul(ps_qqT[:], sb_96[:, C_WQ2:C_WQ2 + 96], sb_qT[:])
        sb_qqT = sbt([96, 1], "sb_qqT")
        nc.scalar.copy(sb_qqT[:], ps_qqT[:])
        sb_v2T = sbt([96, 8], "sb_v2T")
        nc.scalar.copy(sb_v2T[:], ps_v2T[:])
        sb_tmp = sbt([96, 8], "sb_tmp")
        nc.scalar.mul(sb_tmp[:], ps_k2T[:], sb_qqT[:])
        ps_s2 = pst([16, 8], "ps_s2")
        nc.tensor.matmul(ps_s2[:], sb_96[:, C_E:C_E + 16], sb_tmp[:])
        sb_e2 = sbt([16, 8], "sb_e2")
        nc.scalar.activation(sb_e2[:], ps_s2[:], ACT.Exp, scale=SCL)
        sb_sum2 = sbt([16, 1], "sb_sum2")
        nc.vector.tensor_reduce(sb_sum2[:], sb_e2[:], AX.X, ALU.add)
        sb_rec2 = sbt([16, 1], "sb_rec2")
        nc.vector.reciprocal(sb_rec2[:], sb_sum2[:])
        sb_a2 = sbt([16, 8], "sb_a2")
        nc.scalar.mul(sb_a2[:], sb_e2[:], sb_rec2[:])
        ps_a2e = pst([96, 8], "ps_a2e")
        nc.tensor.matmul(ps_a2e[:], sb_16[:], sb_a2[:])
        sb_scr = sbt([96, 8], "sb_scr")
        nc.vector.tensor_mul(sb_scr[:], ps_a2e[:], sb_v2T[:])
        sb_o2T = sbt([96, 1], "sb_o2T")
        nc.vector.tensor_reduce(sb_o2T[:], sb_scr[:], AX.X, ALU.add)
        sb_ot1 = sbt([96, 1], "sb_ot1")
        nc.vector.tensor_add(sb_ot1[:], sb_o2T[:], sb_qqT[:])
        ps_r2 = pst([96, 1], "ps_r2")
        nc.tensor.matmul(ps_r2[:], sb_96[:, C_F2:C_F2 + 96], sb_ot1[:])
        sb_r2 = sbt([96, 1], "sb_r2")
        nc.scalar.activation(sb_r2[:], ps_r2[:], ACT.Relu)
        sb_otf = sbt([96, 1], "sb_otf")
        nc.vector.tensor_add(sb_otf[:], sb_ot1[:], sb_r2[:])

        # ---- tail: trans = O@Wo, de = (De*gate).sum, alpha, FiLM mix ----
        ps_tr = pst([128, 4], "ps_tr")
        for m in range(4):
            nc.tensor.matmul(ps_tr[:, m:m + 1],
                             sb_96[:, C_WO + 128 * m:C_WO + 128 * m + 128],
                             sb_otf[:])
        ps_de = pst([128, 4], "ps_de")
        for m in range(4):
            nc.tensor.matmul(ps_de[:, m:m + 1],
                             sb_8[:, 128 * m:128 * m + 128],
                             sb_8[:, C_GATE:C_GATE + 1])
        sb_de = sbt([128, 4], "sb_de")
        nc.scalar.copy(sb_de[:], ps_de[:])
        ps_al = pst([128, 4], "ps_al")
        for m in range(4):
            for k in range(4):
                nc.tensor.matmul(ps_al[:, m:m + 1],
                                 sb_wa[:, 512 * k + 128 * m:512 * k + 128 * m + 128],
                                 xT_blk(k), start=(k == 0), stop=(k == 3))
        sb_az = sbt([128, 4], "sb_az")
        nc.vector.tensor_add(sb_az[:], ps_al[:], sb_t[:, 0:4])
        sb_al = sbt([128, 4], "sb_al")
        nc.scalar.activation(sb_al[:], sb_az[:], ACT.Sigmoid)
        sb_d1 = sbt([128, 4], "sb_d1")
        nc.vector.tensor_sub(sb_d1[:], ps_tr[:], sb_de[:])
        sb_d2 = sbt([128, 4], "sb_d2")
        nc.vector.tensor_mul(sb_d2[:], sb_d1[:], sb_al[:])
        sb_mx = sbt([128, 4], "sb_mx")
        nc.vector.tensor_add(sb_mx[:], sb_d2[:], sb_de[:])
        sb_sc = sbt([128, 4], "sb_sc")
        nc.vector.tensor_mul(sb_sc[:], sb_mx[:], sb_t[:, 4:8])
        sb_out = sbt([128, 4], "sb_out")
        nc.vector.tensor_scalar_add(sb_out[:], sb_sc[:], sb_t[:, 8:9])

        nc.sync.dma_start(out=d_out[:], in_=sb_out[:])

    nc.compile()
    return nc


def _to_chunks128(a, cols):
    """(512, cols) -> (128, 4*cols) with column block k = rows [128k, 128k+128)."""
    return np.ascontiguousarray(
        a.reshape(4, 128, cols).transpose(1, 0, 2).reshape(128, 4 * cols),
        dtype=np.float32)


def _pack_inputs(inputs):
    gate = np.asarray(inputs['gate'], np.float32)
    x = np.asarray(inputs['x'], np.float32)
    Wa = np.asarray(inputs['Wa'], np.float32)
    ba = np.asarray(inputs['ba'], np.float32)
    Wqr = np.asarray(inputs['Wqr'], np.float32)
    bqr = np.asarray(inputs['bqr'], np.float32)
    P = np.asarray(inputs['P'], np.float32)
    De = np.asarray(inputs['De'], np.float32)
    regs = np.asarray(inputs['regs'], np.float32)

    wa_p = _to_chunks128(Wa, 512)
    wqr_p = _to_chunks128(Wqr, 96)
    xT4 = np.ascontiguousarray(x.reshape(4, 128).T, dtype=np.float32)
    baT4 = np.ascontiguousarray(ba.reshape(4, 128).T, dtype=np.float32)

    E = np.zeros((96, 16), np.float32)
    E[np.arange(96), np.arange(96) // 6] = 1.0
    b16 = np.ascontiguousarray(E.T)
    I8 = np.eye(8, dtype=np.float32)

    in_maps = []
    for i in range(NPROC):
        b, t = i // 4, i % 4
        px = np.concatenate([_to_chunks128(P[b, t], 8), xT4], axis=1)
        w1 = np.concatenate([
            _to_chunks128(np.asarray(inputs['Wq1'], np.float32)[i], 96),
            _to_chunks128(np.asarray(inputs['Wk1'], np.float32)[i], 96),
            _to_chunks128(np.asarray(inputs['Wv1'], np.float32)[i], 96),
        ], axis=1)
        b96 = np.concatenate([
            np.asarray(inputs['fc1'], np.float32)[i],
            np.asarray(inputs['Wq2'], np.float32)[i],
            np.asarray(inputs['Wk2'], np.float32)[i],
            np.asarray(inputs['Wv2'], np.float32)[i],
            np.asarray(inputs['fc2'], np.float32)[i],
            np.asarray(inputs['Wo'], np.float32)[i],
            E,
            bqr.reshape(96, 1),
        ], axis=1)
        b8 = np.concatenate([
            De[b, t].T,
            gate.reshape(8, 1),
            I8,
        ], axis=1)
        offs = 1.0 if t in (0, 2) else 0.0
        t128 = np.concatenate([
            baT4,
            np.ascontiguousarray(regs[b, t].reshape(4, 128).T),
            np.full((128, 1), offs, np.float32),
        ], axis=1)
        in_maps.append({
            'px': np.ascontiguousarray(px),
            'w1': np.ascontiguousarray(w1),
            'b96': np.ascontiguousarray(b96),
            'wqr': wqr_p,
            'b8': np.ascontiguousarray(b8),
            'b16': b16,
            't128': np.ascontiguousarray(t128),
            'wa': wa_p,
        })
    return in_maps


def _run(inputs, trace=False):
    from concourse.bass_utils import run_bass_kernel_spmd
    if 'nc' not in _CACHE:
        _CACHE['nc'] = _build_nc()
    nc = _CACHE['nc']
    in_maps = _pack_inputs(inputs)
    res = run_bass_kernel_spmd(nc, in_maps, list(range(NPROC)), trace=trace)
    out = np.zeros((NB, 4, NM), np.float32)
    for i in range(NPROC):
        out[i // 4, i % 4] = np.asarray(res.results[i]['out']).T.reshape(NM)
    return out, res


def kernel(**inputs):
    out, _ = _run(inputs, trace=False)
    return out
